# revision 1
# baseline (speedup 1.0000x reference)
"""Trainium2 Bass kernel: batched scaled-dot-product attention.

reference: out[b] = softmax(scale * x1[b] @ x2[b].T) @ x3[b]
shapes: x1,x2,x3 = [16, 2048, 128] fp32.

Sharding: B=16 batches data-parallel over 8 NeuronCores (2 batches/core).

Device algorithm (per batch, per q-half of 1024):
  for each k-chunk (128 rows of K):
    S^T[k_chunk, q] = (K^T chunk).T-matmul(Q^T)          TensorE, float32r
    eS = exp(scale * S^T)            PSUM -> SBUF(f32r)  ScalarE
    outT[dv, q] += (V chunk)-stationary matmul(eS)       TensorE, accumulate in PSUM
    sums accumulation split: GPSIMD (chunks 0-6) / DVE (chunks 7-15)
  sums-bcast = ones-matmul(acc);  rcp = 1/sums (DVE approx)
  out^T = outT * rcp  -> DMA out.
Host: pre-transposes Q,K (and interleaves V), pre-rounds to float32r (e8m11),
post-transposes the output. All heavy math runs on device.
"""
import os
import sys
import types
import numpy as np
from contextlib import ExitStack

import concourse.bass as bass  # noqa: F401
from concourse import bacc
import concourse.mybir as mybir
import concourse.tile as tile
from concourse.bass_utils import run_bass_kernel_spmd

f32 = mybir.dt.float32
f32r = mybir.dt.float32r

B, SQ, SK, D = 16, 2048, 2048, 128
NCORES = 8
BPC = B // NCORES  # batches per core
KC = SK // 128     # k chunks
NH = 2             # q halves
HW_ = SQ // NH     # 1024


def _install_ntff_hook():
    """Register the axon NTFF profile hook (used only when tracing)."""
    try:
        from antenv import axon_hooks  # noqa: F401
        return
    except ImportError:
        pass
    try:
        m = types.ModuleType("antenv.axon_hooks")
        m._hook = None
        m.set_axon_ntff_profile_hook = lambda h: setattr(m, "_hook", h)
        m.get_axon_ntff_profile_hook = lambda: m._hook
        sys.modules["antenv.axon_hooks"] = m
        import antenv
        antenv.axon_hooks = m
        from trn_agent_boot.trn_boot import _ntff_profile_via_ctypes
        m._hook = _ntff_profile_via_ctypes("/opt/axon/libaxon_pjrt.so")
    except Exception:
        pass


def round_fp32r(x: np.ndarray) -> np.ndarray:
    """RNE round fp32 -> float32r (e8m11: drop low 12 mantissa bits)."""
    u = np.ascontiguousarray(x, dtype=np.float32).view(np.uint32).astype(np.uint64)
    keep = 12
    lsb = (u >> keep) & 1
    bias = (1 << (keep - 1)) - 1 + lsb
    r = ((u + bias) & ~np.uint64((1 << keep) - 1)).astype(np.uint32)
    return r.view(np.float32)


def build(scale: float):
    nc = bacc.Bacc("TRN2", target_bir_lowering=False, debug=False)
    qt = nc.dram_tensor("qt", [BPC, 128, SQ], f32r, kind="ExternalInput")
    kt = nc.dram_tensor("kt", [BPC, 128, SK], f32r, kind="ExternalInput")
    vv = nc.dram_tensor("v", [BPC, 128, SK], f32r, kind="ExternalInput")
    ones = nc.dram_tensor("ones", [128, 128], f32r, kind="ExternalInput")
    oo = nc.dram_tensor("o", [BPC, 128, SQ], f32, kind="ExternalOutput")

    Exp = mybir.ActivationFunctionType.Exp

    with tile.TileContext(nc) as tc, ExitStack() as ctx:
        inp = ctx.enter_context(tc.tile_pool(name="inp", bufs=2))
        es_pool = ctx.enter_context(tc.tile_pool(name="es", bufs=12))
        acc_pool = ctx.enter_context(tc.tile_pool(name="acc", bufs=2))
        out_pool = ctx.enter_context(tc.tile_pool(name="out", bufs=2))
        cpool = ctx.enter_context(tc.tile_pool(name="const", bufs=1))
        psS = ctx.enter_context(tc.tile_pool(name="psS", bufs=2, space="PSUM"))
        psO = ctx.enter_context(tc.tile_pool(name="psO", bufs=2, space="PSUM"))

        ones_sb = cpool.tile([128, 128], f32r, tag="ones")
        nc.sync.dma_start(ones_sb[:], ones.ap())

        for b in range(BPC):
            qt_sb = inp.tile([128, SQ], f32r, tag="qt")
            kt_sb = inp.tile([128, SK], f32r, tag="kt")
            v_sb = inp.tile([128, SK], f32r, tag="v")
            nc.sync.dma_start(qt_sb[:], qt.ap()[b])
            nc.sync.dma_start(kt_sb[:], kt.ap()[b])
            nc.sync.dma_start(v_sb[:], vv.ap()[b])
            ot_sb = out_pool.tile([128, SQ], f32, tag="ot")

            for h in range(NH):
                q0 = h * HW_
                ps_o = psO.tile([128, HW_], f32, tag="psO")
                acc_g = acc_pool.tile([128, HW_], f32, tag="accg")
                acc_v = acc_pool.tile([128, HW_], f32, tag="accv")
                es_prev = None
                for k in range(KC):
                    ps_s = psS.tile([128, HW_], f32, tag="S")
                    for j in range(HW_ // 512):
                        nc.tensor.matmul(
                            ps_s[:, j * 512:(j + 1) * 512],
                            kt_sb[:, k * 128:(k + 1) * 128],
                            qt_sb[:, q0 + j * 512:q0 + (j + 1) * 512],
                            start=True, stop=True,
                        )
                    es = es_pool.tile([128, HW_], f32r, tag="es")
                    nc.scalar.activation(es[:], ps_s[:], Exp, scale=scale)
                    for j in range(HW_ // 512):
                        nc.tensor.matmul(
                            ps_o[:, j * 512:(j + 1) * 512],
                            v_sb[:, k * 128:(k + 1) * 128],
                            es[:, j * 512:(j + 1) * 512],
                            start=(k == 0), stop=(k == KC - 1),
                        )
                    esf = es[:].bitcast(f32)
                    if k == 1:
                        nc.gpsimd.tensor_add(acc_g[:], es_prev, esf)
                    elif 2 <= k <= 6:
                        nc.gpsimd.tensor_add(acc_g[:], acc_g[:], esf)
                    elif k == 8:
                        nc.vector.tensor_add(acc_v[:], es_prev, esf)
                    elif k >= 9:
                        nc.vector.tensor_add(acc_v[:], acc_v[:], esf)
                    es_prev = esf

                acc_r = acc_pool.tile([128, HW_], f32r, tag="accr")
                nc.vector.tensor_add(acc_r[:], acc_v[:], acc_g[:])
                ps_b = psS.tile([128, HW_], f32, tag="S")
                for j in range(HW_ // 512):
                    nc.tensor.matmul(
                        ps_b[:, j * 512:(j + 1) * 512],
                        ones_sb[:],
                        acc_r[:, j * 512:(j + 1) * 512],
                        start=True, stop=True,
                    )
                rcp = acc_pool.tile([128, HW_], f32, tag="rcp")
                nc.vector.reciprocal_approx_fast(out=rcp[:], in_=ps_b[:])
                nc.vector.tensor_mul(ot_sb[:, q0:q0 + HW_], ps_o[:], rcp[:])

            nc.sync.dma_start(oo.ap()[b], ot_sb[:])

    nc.compile()
    return nc


_BUILD_CACHE = {}


def _get_nc(scale: float):
    key = round(float(scale), 9)
    if key not in _BUILD_CACHE:
        _BUILD_CACHE[key] = build(float(scale))
    return _BUILD_CACHE[key]


def kernel(x1, x2, x3, x4=None, scale_factor=None, **_ignored):
    x1 = np.asarray(x1, dtype=np.float32)
    x2 = np.asarray(x2, dtype=np.float32)
    x3 = np.asarray(x3, dtype=np.float32)
    scale = float(np.asarray(scale_factor).reshape(-1)[0])

    # host prep: transpose Q,K to [d, s]; interleave V rows to [p, c*d]; round to f32r
    qt = round_fp32r(x1.transpose(0, 2, 1))                     # [B, 128, SQ]
    kt = round_fp32r(x2.transpose(0, 2, 1))                     # [B, 128, SK]
    v = round_fp32r(
        x3.reshape(B, KC, 128, D).transpose(0, 2, 1, 3).reshape(B, 128, KC * D)
    )                                                           # [B, 128, SK]
    ones = np.ones((128, 128), dtype=np.float32)

    nc = _get_nc(scale)
    in_maps = []
    for c in range(NCORES):
        s = slice(c * BPC, (c + 1) * BPC)
        in_maps.append({
            "qt": np.ascontiguousarray(qt[s]),
            "kt": np.ascontiguousarray(kt[s]),
            "v": np.ascontiguousarray(v[s]),
            "ones": ones,
        })

    trace = bool(int(os.environ.get("KERNEL_TRACE", "0")))
    kwargs = {}
    if trace:
        _install_ntff_hook()
        os.environ["BASS_PERFETTO_PROFILE_ALL_CORES"] = "1"
        kwargs = dict(trace=True, trace_kwargs={"title": "attention"})
    res = run_bass_kernel_spmd(nc, in_maps, core_ids=list(range(NCORES)), **kwargs)
    if trace:
        kernel.last_exec_ns = res.exec_time_ns
        kernel.last_trace = res.instructions_and_trace
        kernel.last_mean_exec_ns = res.mean_exec_time_ns

    outT = np.stack([r["o"] for r in res.results])              # [8, BPC, 128, SQ]
    out = outT.reshape(B, 128, SQ).transpose(0, 2, 1)           # [B, SQ, 128]
    return np.ascontiguousarray(out, dtype=np.float32)


kernel.last_exec_ns = None
kernel.last_trace = None
kernel.last_mean_exec_ns = None


# revision 3
# speedup vs baseline: 1.0776x; 1.0776x over previous
"""Trainium2 Bass kernel: batched scaled-dot-product attention.

reference: out[b] = softmax(scale * x1[b] @ x2[b].T) @ x3[b]
shapes: x1,x2,x3 = [16, 2048, 128] fp32.

Sharding: B=16 batches data-parallel over 8 NeuronCores (2 batches/core).

Device algorithm (per batch, per q-half of 1024):
  for k-chunk in 16 (128 K-rows each):
    S^T[k, q]   = matmul(lhsT=K^T chunk, rhs=Q^T half)    TensorE float32r
    eS          = exp(scale * S^T)     PSUM -> SBUF f32r  ScalarE
    outT[dv,q] += matmul(lhsT=V chunk, rhs=eS)            TensorE accumulate
    softmax denominators: partial sums of eS chunks accumulated on
    GPSIMD (chunks 0-5) and DVE (chunks 6-15)
  sums = ones-matmul(acc_g) + ones-matmul(acc_v)  (PSUM accumulate, bcast to
  all partitions);  rcp = approx 1/sums (DVE);  out^T = outT * rcp -> DMA.

Host side does layout only: pre-transpose Q,K; interleave V rows; pre-round
inputs to float32r (e8m11, bit-identical to the device cast); transpose the
output back. All FLOPs run on device.
"""
import os
import sys
import types
import numpy as np
from contextlib import ExitStack

import concourse.bass as bass  # noqa: F401
from concourse import bacc
import concourse.mybir as mybir
import concourse.tile as tile
import concourse.bass_utils as bass_utils
from concourse.bass_utils import run_bass_kernel_spmd

f32 = mybir.dt.float32
f32r = mybir.dt.float32r

B, SQ, SK, D = 16, 2048, 2048, 128
NCORES = 8
BPC = B // NCORES  # batches per core
KC = SK // 128     # k chunks
NH = 2             # q halves
HW_ = SQ // NH     # 1024
GP_CHUNKS = 6      # sums chunks 0..5 on GPSIMD, rest on DVE


def _patch_ldw_opt():
    """Enable walrus LDWEIGHTS optimization (background weight-buffer overlap)."""
    if getattr(bass_utils, "_ldw_patched", False):
        return
    if not bool(int(os.environ.get("KERNEL_LDWOPT", "1"))):
        return
    orig = bass_utils.run_command

    def patched(argv, **kw):
        if isinstance(argv, list):
            argv = [
                "--enable-ldw-opt=true" if a == "--enable-ldw-opt=false" else a
                for a in argv
            ]
        return orig(argv, **kw)

    bass_utils.run_command = patched
    bass_utils._ldw_patched = True


def _install_ntff_hook():
    """Register the axon NTFF profile hook (used only when tracing)."""
    try:
        from antenv import axon_hooks  # noqa: F401
        return
    except ImportError:
        pass
    try:
        m = types.ModuleType("antenv.axon_hooks")
        m._hook = None
        m.set_axon_ntff_profile_hook = lambda h: setattr(m, "_hook", h)
        m.get_axon_ntff_profile_hook = lambda: m._hook
        sys.modules["antenv.axon_hooks"] = m
        import antenv
        antenv.axon_hooks = m
        from trn_agent_boot.trn_boot import _ntff_profile_via_ctypes
        m._hook = _ntff_profile_via_ctypes("/opt/axon/libaxon_pjrt.so")
    except Exception:
        pass


def round_fp32r(x: np.ndarray) -> np.ndarray:
    """RNE round fp32 -> float32r (e8m11: drop low 12 mantissa bits)."""
    u = np.ascontiguousarray(x, dtype=np.float32).view(np.uint32).astype(np.uint64)
    keep = 12
    lsb = (u >> keep) & 1
    bias = (1 << (keep - 1)) - 1 + lsb
    r = ((u + bias) & ~np.uint64((1 << keep) - 1)).astype(np.uint32)
    return r.view(np.float32)


def build(scale: float):
    _patch_ldw_opt()
    nc = bacc.Bacc("TRN2", target_bir_lowering=False, debug=False)
    qt = nc.dram_tensor("qt", [BPC, 128, SQ], f32r, kind="ExternalInput")
    kt = nc.dram_tensor("kt", [BPC, 128, SK], f32r, kind="ExternalInput")
    vv = nc.dram_tensor("v", [BPC, 128, SK], f32r, kind="ExternalInput")
    ones = nc.dram_tensor("ones", [128, 128], f32r, kind="ExternalInput")
    oo = nc.dram_tensor("o", [BPC, 128, SQ], f32, kind="ExternalOutput")

    Exp = mybir.ActivationFunctionType.Exp

    with tile.TileContext(nc) as tc, ExitStack() as ctx:
        inp = ctx.enter_context(tc.tile_pool(name="inp", bufs=2))
        es_pool = ctx.enter_context(tc.tile_pool(name="es", bufs=12))
        acc_pool = ctx.enter_context(tc.tile_pool(name="acc", bufs=2))
        out_pool = ctx.enter_context(tc.tile_pool(name="out", bufs=2))
        cpool = ctx.enter_context(tc.tile_pool(name="const", bufs=1))
        psS = ctx.enter_context(tc.tile_pool(name="psS", bufs=2, space="PSUM"))
        psO = ctx.enter_context(tc.tile_pool(name="psO", bufs=2, space="PSUM"))

        ones_sb = cpool.tile([128, 128], f32r, tag="ones")
        nc.sync.dma_start(ones_sb[:], ones.ap())

        for b in range(BPC):
            qt_sb = inp.tile([128, SQ], f32r, tag="qt")
            kt_sb = inp.tile([128, SK], f32r, tag="kt")
            v_sb = inp.tile([128, SK], f32r, tag="v")
            # chunked loads so the first chunks' operands arrive early
            for h in range(NH):
                nc.sync.dma_start(
                    qt_sb[:, h * HW_:(h + 1) * HW_], qt.ap()[b][:, h * HW_:(h + 1) * HW_]
                )
            for k in range(KC):
                sl = slice(k * 128, (k + 1) * 128)
                nc.sync.dma_start(kt_sb[:, sl], kt.ap()[b][:, sl])
            for k in range(KC):
                sl = slice(k * 128, (k + 1) * 128)
                nc.sync.dma_start(v_sb[:, sl], vv.ap()[b][:, sl])
            ot_sb = out_pool.tile([128, SQ], f32, tag="ot")

            for h in range(NH):
                q0 = h * HW_
                ps_o = psO.tile([128, HW_], f32, tag="psO")
                acc_g = acc_pool.tile([128, HW_], f32, tag="accg")
                acc_gr = acc_pool.tile([128, HW_], f32r, tag="accgr")
                acc_v = acc_pool.tile([128, HW_], f32, tag="accv")
                acc_vr = acc_pool.tile([128, HW_], f32r, tag="accvr")
                es_prev = None
                for k in range(KC):
                    ps_s = psS.tile([128, HW_], f32, tag="S")
                    for j in range(HW_ // 512):
                        nc.tensor.matmul(
                            ps_s[:, j * 512:(j + 1) * 512],
                            kt_sb[:, k * 128:(k + 1) * 128],
                            qt_sb[:, q0 + j * 512:q0 + (j + 1) * 512],
                            start=True, stop=True,
                        )
                    es = es_pool.tile([128, HW_], f32r, tag="es")
                    nc.scalar.activation(es[:], ps_s[:], Exp, scale=scale)
                    for j in range(HW_ // 512):
                        nc.tensor.matmul(
                            ps_o[:, j * 512:(j + 1) * 512],
                            v_sb[:, k * 128:(k + 1) * 128],
                            es[:, j * 512:(j + 1) * 512],
                            start=(k == 0), stop=(k == KC - 1),
                        )
                    esf = es[:].bitcast(f32)
                    if k == 0:
                        pass
                    elif k < GP_CHUNKS:
                        if k == 1:
                            nc.gpsimd.tensor_add(acc_g[:], es_prev, esf)
                        else:
                            nc.gpsimd.tensor_add(acc_g[:], acc_g[:], esf)
                    else:
                        if k == GP_CHUNKS:
                            # cast GPSIMD partial to f32r for the ones-matmul;
                            # chunk GP_CHUNKS itself is es_prev at the next k
                            nc.vector.tensor_copy(acc_gr[:], acc_g[:])
                        elif k == GP_CHUNKS + 1:
                            nc.vector.tensor_add(acc_v[:], es_prev, esf)
                        elif k == KC - 1:
                            nc.vector.tensor_add(acc_vr[:], acc_v[:], esf)
                        else:
                            nc.vector.tensor_add(acc_v[:], acc_v[:], esf)
                    es_prev = esf

                # broadcast column sums to all partitions via ones-matmul,
                # accumulating the GPSIMD part (ready mid-half) + DVE part.
                ps_b = psO.tile([128, HW_], f32, tag="psO")
                for j in range(HW_ // 512):
                    nc.tensor.matmul(
                        ps_b[:, j * 512:(j + 1) * 512],
                        ones_sb[:],
                        acc_gr[:, j * 512:(j + 1) * 512],
                        start=True, stop=False,
                    )
                for j in range(HW_ // 512):
                    nc.tensor.matmul(
                        ps_b[:, j * 512:(j + 1) * 512],
                        ones_sb[:],
                        acc_vr[:, j * 512:(j + 1) * 512],
                        start=False, stop=True,
                    )
                rcp = acc_pool.tile([128, HW_], f32, tag="rcp")
                nc.vector.reciprocal_approx_fast(out=rcp[:], in_=ps_b[:])
                # drain psO early (frees the slot for the next half), then
                # normalize in place in SBUF
                osl = ot_sb[:, q0:q0 + HW_]
                nc.vector.tensor_copy(osl, ps_o[:])
                nc.vector.tensor_mul(osl, osl, rcp[:])

            nc.sync.dma_start(oo.ap()[b], ot_sb[:])

    nc.compile()
    return nc


_BUILD_CACHE = {}


def _get_nc(scale: float):
    key = round(float(scale), 9)
    if key not in _BUILD_CACHE:
        _BUILD_CACHE[key] = build(float(scale))
    return _BUILD_CACHE[key]


def kernel(x1, x2, x3, x4=None, scale_factor=None, **_ignored):
    x1 = np.asarray(x1, dtype=np.float32)
    x2 = np.asarray(x2, dtype=np.float32)
    x3 = np.asarray(x3, dtype=np.float32)
    scale = float(np.asarray(scale_factor).reshape(-1)[0])

    # host prep: transpose Q,K to [d, s]; interleave V rows to [p, c*d]; round f32r
    qt = round_fp32r(x1.transpose(0, 2, 1))                     # [B, 128, SQ]
    kt = round_fp32r(x2.transpose(0, 2, 1))                     # [B, 128, SK]
    v = round_fp32r(
        x3.reshape(B, KC, 128, D).transpose(0, 2, 1, 3).reshape(B, 128, KC * D)
    )                                                           # [B, 128, SK]
    ones = np.ones((128, 128), dtype=np.float32)

    nc = _get_nc(scale)
    in_maps = []
    for c in range(NCORES):
        s = slice(c * BPC, (c + 1) * BPC)
        in_maps.append({
            "qt": np.ascontiguousarray(qt[s]),
            "kt": np.ascontiguousarray(kt[s]),
            "v": np.ascontiguousarray(v[s]),
            "ones": ones,
        })

    trace = bool(int(os.environ.get("KERNEL_TRACE", "0")))
    kwargs = {}
    if trace:
        _install_ntff_hook()
        if bool(int(os.environ.get("KERNEL_TRACE_ALL", "0"))):
            os.environ["BASS_PERFETTO_PROFILE_ALL_CORES"] = "1"
        kwargs = dict(trace=True, trace_kwargs={"title": "attention"})
    res = run_bass_kernel_spmd(nc, in_maps, core_ids=list(range(NCORES)), **kwargs)
    if trace:
        kernel.last_exec_ns = res.exec_time_ns
        kernel.last_trace = res.instructions_and_trace
        kernel.last_mean_exec_ns = res.mean_exec_time_ns

    outT = np.stack([r["o"] for r in res.results])              # [8, BPC, 128, SQ]
    out = outT.reshape(B, 128, SQ).transpose(0, 2, 1)           # [B, SQ, 128]
    return np.ascontiguousarray(out, dtype=np.float32)


kernel.last_exec_ns = None
kernel.last_trace = None
kernel.last_mean_exec_ns = None


# revision 4
# speedup vs baseline: 1.0888x; 1.0104x over previous
"""Trainium2 Bass kernel: batched scaled-dot-product attention.

reference: out[b] = softmax(scale * x1[b] @ x2[b].T) @ x3[b]
shapes: x1,x2,x3 = [16, 2048, 128] fp32.

Sharding: B=16 batches data-parallel over 8 NeuronCores (2 batches/core).

Device algorithm (per batch, per q-half of 1024):
  for k-chunk in 16 (128 K-rows each):
    S^T[k, q]   = matmul(lhsT=K^T chunk, rhs=Q^T half)    TensorE float32r
    eS          = exp(scale * S^T)     PSUM -> SBUF f32r  ScalarE
    outT[dv,q] += matmul(lhsT=V chunk, rhs=eS)            TensorE accumulate
    softmax denominators: partial sums of eS chunks accumulated on
    GPSIMD (chunks 0-5) and DVE (chunks 6-15)
  sums = ones-matmul(acc_g) + ones-matmul(acc_v)  (PSUM accumulate, bcast to
  all partitions);  rcp = approx 1/sums (DVE);  out^T = outT * rcp -> DMA.

Host side does layout only: pre-transpose Q,K; interleave V rows; pre-round
inputs to float32r (e8m11, bit-identical to the device cast); transpose the
output back. All FLOPs run on device.
"""
import os
import sys
import types
import numpy as np
from contextlib import ExitStack

import concourse.bass as bass  # noqa: F401
from concourse import bacc
import concourse.mybir as mybir
import concourse.tile as tile
import concourse.bass_utils as bass_utils
from concourse.bass_utils import run_bass_kernel_spmd

f32 = mybir.dt.float32
f32r = mybir.dt.float32r

B, SQ, SK, D = 16, 2048, 2048, 128
NCORES = 8
BPC = B // NCORES  # batches per core
KC = SK // 128     # k chunks
NH = 2             # q halves
HW_ = SQ // NH     # 1024
GP_CHUNKS = 6      # sums chunks 0..5 on GPSIMD, rest on DVE


def _patch_ldw_opt():
    """Enable walrus LDWEIGHTS optimization (background weight-buffer overlap)."""
    if getattr(bass_utils, "_ldw_patched", False):
        return
    if not bool(int(os.environ.get("KERNEL_LDWOPT", "1"))):
        return
    orig = bass_utils.run_command

    def patched(argv, **kw):
        if isinstance(argv, list):
            argv = [
                "--enable-ldw-opt=true" if a == "--enable-ldw-opt=false" else a
                for a in argv
            ]
        return orig(argv, **kw)

    bass_utils.run_command = patched
    bass_utils._ldw_patched = True


def _install_ntff_hook():
    """Register the axon NTFF profile hook (used only when tracing)."""
    try:
        from antenv import axon_hooks  # noqa: F401
        return
    except ImportError:
        pass
    try:
        m = types.ModuleType("antenv.axon_hooks")
        m._hook = None
        m.set_axon_ntff_profile_hook = lambda h: setattr(m, "_hook", h)
        m.get_axon_ntff_profile_hook = lambda: m._hook
        sys.modules["antenv.axon_hooks"] = m
        import antenv
        antenv.axon_hooks = m
        from trn_agent_boot.trn_boot import _ntff_profile_via_ctypes
        m._hook = _ntff_profile_via_ctypes("/opt/axon/libaxon_pjrt.so")
    except Exception:
        pass


def round_fp32r(x: np.ndarray) -> np.ndarray:
    """RNE round fp32 -> float32r (e8m11: drop low 12 mantissa bits)."""
    u = np.ascontiguousarray(x, dtype=np.float32).view(np.uint32).astype(np.uint64)
    keep = 12
    lsb = (u >> keep) & 1
    bias = (1 << (keep - 1)) - 1 + lsb
    r = ((u + bias) & ~np.uint64((1 << keep) - 1)).astype(np.uint32)
    return r.view(np.float32)


def build(scale: float):
    _patch_ldw_opt()
    nc = bacc.Bacc("TRN2", target_bir_lowering=False, debug=False)
    qt = nc.dram_tensor("qt", [BPC, 128, SQ], f32r, kind="ExternalInput")
    kt = nc.dram_tensor("kt", [BPC, 128, SK], f32r, kind="ExternalInput")
    vv = nc.dram_tensor("v", [BPC, 128, SK], f32r, kind="ExternalInput")
    ones = nc.dram_tensor("ones", [128, 128], f32r, kind="ExternalInput")
    oo = nc.dram_tensor("o", [BPC, 128, SQ], f32, kind="ExternalOutput")

    Exp = mybir.ActivationFunctionType.Exp

    with tile.TileContext(nc) as tc, ExitStack() as ctx:
        inp = ctx.enter_context(tc.tile_pool(name="inp", bufs=2))
        es_pool = ctx.enter_context(tc.tile_pool(name="es", bufs=18))
        acc_pool = ctx.enter_context(tc.tile_pool(name="acc", bufs=2))
        out_pool = ctx.enter_context(tc.tile_pool(name="out", bufs=2))
        cpool = ctx.enter_context(tc.tile_pool(name="const", bufs=1))
        psS = ctx.enter_context(tc.tile_pool(name="psS", bufs=2, space="PSUM"))
        psO = ctx.enter_context(tc.tile_pool(name="psO", bufs=2, space="PSUM"))

        ones_sb = cpool.tile([128, 128], f32r, tag="ones")
        nc.sync.dma_start(ones_sb[:], ones.ap())

        for b in range(BPC):
            qt_sb = inp.tile([128, SQ], f32r, tag="qt")
            kt_sb = inp.tile([128, SK], f32r, tag="kt")
            v_sb = inp.tile([128, SK], f32r, tag="v")
            # chunked loads, critical-first so chunk 0 can start ASAP
            nc.sync.dma_start(qt_sb[:, 0:HW_], qt.ap()[b][:, 0:HW_])
            for k in range(KC):
                sl = slice(k * 128, (k + 1) * 128)
                nc.sync.dma_start(kt_sb[:, sl], kt.ap()[b][:, sl])
                nc.sync.dma_start(v_sb[:, sl], vv.ap()[b][:, sl])
            nc.sync.dma_start(qt_sb[:, HW_:SQ], qt.ap()[b][:, HW_:SQ])
            ot_sb = out_pool.tile([128, SQ], f32, tag="ot")

            for h in range(NH):
                q0 = h * HW_
                ps_o = psO.tile([128, HW_], f32, tag="psO")
                acc_g = acc_pool.tile([128, HW_], f32, tag="accg")
                acc_gr = acc_pool.tile([128, HW_], f32r, tag="accgr")
                acc_v = acc_pool.tile([128, HW_], f32, tag="accv")
                acc_vr = acc_pool.tile([128, HW_], f32r, tag="accvr")
                es_prev = None
                for k in range(KC):
                    ps_s = psS.tile([128, HW_], f32, tag="S")
                    for j in range(HW_ // 512):
                        nc.tensor.matmul(
                            ps_s[:, j * 512:(j + 1) * 512],
                            kt_sb[:, k * 128:(k + 1) * 128],
                            qt_sb[:, q0 + j * 512:q0 + (j + 1) * 512],
                            start=True, stop=True,
                        )
                    es = es_pool.tile([128, HW_], f32r, tag="es")
                    nc.scalar.activation(es[:], ps_s[:], Exp, scale=scale)
                    for j in range(HW_ // 512):
                        nc.tensor.matmul(
                            ps_o[:, j * 512:(j + 1) * 512],
                            v_sb[:, k * 128:(k + 1) * 128],
                            es[:, j * 512:(j + 1) * 512],
                            start=(k == 0), stop=(k == KC - 1),
                        )
                    esf = es[:].bitcast(f32)
                    if k == 0:
                        pass
                    elif k < GP_CHUNKS:
                        if k == 1:
                            nc.gpsimd.tensor_add(acc_g[:], es_prev, esf)
                        else:
                            nc.gpsimd.tensor_add(acc_g[:], acc_g[:], esf)
                    else:
                        if k == GP_CHUNKS:
                            # cast GPSIMD partial to f32r for the ones-matmul;
                            # chunk GP_CHUNKS itself is es_prev at the next k
                            nc.vector.tensor_copy(acc_gr[:], acc_g[:])
                        elif k == GP_CHUNKS + 1:
                            nc.vector.tensor_add(acc_v[:], es_prev, esf)
                        elif k == KC - 1:
                            nc.vector.tensor_add(acc_vr[:], acc_v[:], esf)
                        else:
                            nc.vector.tensor_add(acc_v[:], acc_v[:], esf)
                    es_prev = esf

                # broadcast column sums to all partitions via ones-matmul,
                # accumulating the GPSIMD part (ready mid-half) + DVE part.
                ps_b = psO.tile([128, HW_], f32, tag="psO")
                for j in range(HW_ // 512):
                    nc.tensor.matmul(
                        ps_b[:, j * 512:(j + 1) * 512],
                        ones_sb[:],
                        acc_gr[:, j * 512:(j + 1) * 512],
                        start=True, stop=False,
                    )
                for j in range(HW_ // 512):
                    nc.tensor.matmul(
                        ps_b[:, j * 512:(j + 1) * 512],
                        ones_sb[:],
                        acc_vr[:, j * 512:(j + 1) * 512],
                        start=False, stop=True,
                    )
                rcp = acc_pool.tile([128, HW_], f32, tag="rcp")
                nc.vector.reciprocal_approx_fast(out=rcp[:], in_=ps_b[:])
                # drain psO early (frees the slot for the next half), then
                # normalize into the DMA staging tile
                ou = acc_pool.tile([128, HW_], f32, tag="ou")
                nc.vector.tensor_copy(ou[:], ps_o[:])
                nc.vector.tensor_mul(ot_sb[:, q0:q0 + HW_], ou[:], rcp[:])

            nc.sync.dma_start(oo.ap()[b], ot_sb[:])

    nc.compile()
    return nc


_BUILD_CACHE = {}


def _get_nc(scale: float):
    key = round(float(scale), 9)
    if key not in _BUILD_CACHE:
        _BUILD_CACHE[key] = build(float(scale))
    return _BUILD_CACHE[key]


def kernel(x1, x2, x3, x4=None, scale_factor=None, **_ignored):
    x1 = np.asarray(x1, dtype=np.float32)
    x2 = np.asarray(x2, dtype=np.float32)
    x3 = np.asarray(x3, dtype=np.float32)
    scale = float(np.asarray(scale_factor).reshape(-1)[0])

    # host prep: transpose Q,K to [d, s]; interleave V rows to [p, c*d]; round f32r
    qt = round_fp32r(x1.transpose(0, 2, 1))                     # [B, 128, SQ]
    kt = round_fp32r(x2.transpose(0, 2, 1))                     # [B, 128, SK]
    v = round_fp32r(
        x3.reshape(B, KC, 128, D).transpose(0, 2, 1, 3).reshape(B, 128, KC * D)
    )                                                           # [B, 128, SK]
    ones = np.ones((128, 128), dtype=np.float32)

    nc = _get_nc(scale)
    in_maps = []
    for c in range(NCORES):
        s = slice(c * BPC, (c + 1) * BPC)
        in_maps.append({
            "qt": np.ascontiguousarray(qt[s]),
            "kt": np.ascontiguousarray(kt[s]),
            "v": np.ascontiguousarray(v[s]),
            "ones": ones,
        })

    trace = bool(int(os.environ.get("KERNEL_TRACE", "0")))
    kwargs = {}
    if trace:
        _install_ntff_hook()
        if bool(int(os.environ.get("KERNEL_TRACE_ALL", "0"))):
            os.environ["BASS_PERFETTO_PROFILE_ALL_CORES"] = "1"
        kwargs = dict(trace=True, trace_kwargs={"title": "attention"})
    res = run_bass_kernel_spmd(nc, in_maps, core_ids=list(range(NCORES)), **kwargs)
    if trace:
        kernel.last_exec_ns = res.exec_time_ns
        kernel.last_trace = res.instructions_and_trace
        kernel.last_mean_exec_ns = res.mean_exec_time_ns

    outT = np.stack([r["o"] for r in res.results])              # [8, BPC, 128, SQ]
    out = outT.reshape(B, 128, SQ).transpose(0, 2, 1)           # [B, SQ, 128]
    return np.ascontiguousarray(out, dtype=np.float32)


kernel.last_exec_ns = None
kernel.last_trace = None
kernel.last_mean_exec_ns = None
